# revision 31
# baseline (speedup 1.0000x reference)
"""LongcatFlashTopkRouter on 8 Trainium2 NeuronCores.

Math (per token t):
    logits = h_t @ W.T                      # [768]
    s      = softmax(logits)
    c      = s + bias                       # bias-corrected selection scores
    idx    = top12(c)                       # descending, ties -> lower index
    w      = 2.5 * s[idx] / sum(s[idx])

Device-side reformulation (per token, no softmax materialization needed):
    e   = exp(logits)           (no max-subtraction: |logits| < ~9 is safe in fp32)
    se  = sum(e)
    z   = e + se * bias         # z has the SAME ordering as c = e/se + bias
    top-16 of z -> (z16, idx16) via DVE max/max_index/match_replace
Host epilogue (cheap, vectorized numpy):
    e16 = z16 - se * bias[idx16]
    w   = 2.5 * e16[:, :12] / sum(e16[:, :12])   # the 1/se factor cancels
Tokens whose top-13 adjacent z-gaps are inside the matmul noise band are
recomputed exactly on the host (same op sequence as the reference).

Matmul runs in fp16 (1 cycle/row on the PE like fp32r, but FWL weight loads
are enabled — fp32-HIGH mode disables them — and DMA bytes halve, which
removes the early-kernel HBM oversubscription from the 6 MB weight preload).
fp16 logit noise (~2.7e-4 std) sits 4x under the 1e-3 risk threshold.

Sharding: tokens (batch*seq = 32768) split evenly across 8 cores (4096 each);
W and bias replicated. Hidden states are pre-transposed on the host into
[tile, k-partition, k-chunk, token] layout so each 128-token tile's 16
contraction chunks are contiguous SBUF-ready blocks.
"""

import numpy as np

import concourse.bass as bass
import concourse.mybir as mybir
from concourse import bacc
from concourse.tile import TileContext
from concourse.bass_utils import run_bass_kernel_spmd

N_CORES = 8
B, S, H, E = 4, 8192, 2048, 768
TOK = B * S // N_CORES      # 4096 tokens per core
TT = 32                     # token tiles of 128 per core
KC = H // 128               # 16 contraction chunks
TOPK = 12
TOP16 = 16
SCALE = 2.5

F32 = mybir.dt.float32
F16 = mybir.dt.float16
F32R = mybir.dt.float32r
U32 = mybir.dt.uint32
EXP = mybir.ActivationFunctionType.Exp
MULT = mybir.AluOpType.mult
ADD = mybir.AluOpType.add

PRO_T = 4  # prologue tiles processed chunk-major while wt streams in


def _prologue_order():
    """Static greedy order of (tile, chunk) matmul pairs for the prologue.

    Uses estimated DMA arrival times (us, relative to first transfer):
    h0/h1 land as two half-tile pieces, h2/h3 whole; wt chunks stream on
    their own queue. Greedy: run whichever pair is ready, else wait for
    the earliest one.
    """
    pair = 0.34
    # Queues are FIFO at ~95 GB/s each: sync = [wt0, h0(c0-3), h0(c4-15)],
    # scalar = [h1, h2, h3], gpsimd = [wt1..wt15] at ~0.95us/chunk.
    # Arrival estimates relative to DMA flow start (~7.4us absolute).
    h_avail = {
        (0, 0): 3.3, (0, 1): 4.3,   # (tile0, chunks 0-7 / 8-15)
        (1, 0): 4.6, (1, 1): 4.6,
        (2, 0): 6.5, (2, 1): 6.5,
        (3, 0): 6.2, (3, 1): 6.2,
    }
    # wt0 leads on the (fast-waking) sync queue; wt1+ ride gpsimd whose first
    # packet flows ~4us in
    wt_avail = [2.3] + [4.0 + 0.72 * c for c in range(1, KC)]
    remaining = [(t, c) for t in range(PRO_T) for c in range(KC)]
    ready_at = {
        (t, c): max(h_avail[(t, 0 if c < 8 else 1)], wt_avail[c])
        for t, c in remaining
    }
    order = []
    pe_t = 0.0
    while remaining:
        ready = [p for p in remaining if ready_at[p] <= pe_t]
        pick = min(ready, key=lambda p: (ready_at[p], p[1], p[0])) if ready else \
            min(remaining, key=lambda p: (ready_at[p], p[1], p[0]))
        remaining.remove(pick)
        order.append(pick)
        pe_t = max(pe_t, ready_at[pick]) + pair
    return order


def build_nc(mm_dtype=F16):
    nc = bacc.Bacc()
    ht = nc.dram_tensor("ht", [TT, 128, KC, 128], mm_dtype, kind="ExternalInput")
    wt = nc.dram_tensor("wt", [128, KC, E], mm_dtype, kind="ExternalInput")
    biasb = nc.dram_tensor("biasb", [128, E], F32, kind="ExternalInput")
    # packed output, one 33-f32 record per (token-in-tile, tile):
    # [z16 f32 | idx16 u32-bits | sumexp f32]; written in ONE end-of-kernel
    # DMA (per-tile 132 B/partition DMAs have terrible completion latency)
    o_pack = nc.dram_tensor("o_pack", [128, TT * 33], F32, kind="ExternalOutput")

    with TileContext(nc) as tc:
        with (
            tc.tile_pool(name="const", bufs=1) as cpool,
            tc.tile_pool(name="hin", bufs=8) as hpool,
            tc.tile_pool(name="mid", bufs=4) as mpool,
            tc.tile_pool(name="ps", bufs=PRO_T, space="PSUM") as ppool,
        ):
            # ---- input DMAs -------------------------------------------------
            # Prologue h tiles split across the sync and scalar queues so they
            # land in parallel (steady-state h rides sync alone: ~82 GB/s
            # demand vs ~260 GB/s the queue sustains). h0 lands as two
            # half-tile pieces so the first matmuls can start early.
            h_tiles = {}
            for t in range(PRO_T):
                h_tiles[t] = hpool.tile(
                    [128, KC * 128], mm_dtype, tag="h", name=f"h_p{t}"
                )
            # wt chunk 0 leads on the sync queue: each DMA queue pays a ~2-3us
            # cold-start before its first packet flows, and gpsimd's is the
            # slowest — wt0 there would gate the first matmul. Remaining wt
            # chunks + bias stream on gpsimd, one tile per chunk so a matmul
            # only waits on its own chunk.
            wt_sb = []
            for c in range(KC):
                wt_sb.append(
                    cpool.tile([128, E], mm_dtype, tag=f"wt{c}", name=f"wt_c{c}")
                )
            nc.sync.dma_start(out=wt_sb[0], in_=wt[:, 0])
            half = KC * 128 // 2
            nc.sync.dma_start(out=h_tiles[0][:, 0:half], in_=ht[0, :, 0:KC // 2])
            nc.sync.dma_start(out=h_tiles[0][:, half:], in_=ht[0, :, KC // 2:])
            nc.scalar.dma_start(out=h_tiles[1], in_=ht[1])
            nc.sync.dma_start(out=h_tiles[3], in_=ht[3])
            nc.scalar.dma_start(out=h_tiles[2], in_=ht[2])
            for c in range(1, KC):
                nc.gpsimd.dma_start(out=wt_sb[c], in_=wt[:, c])
            bias_sb = cpool.tile([128, E], F32)
            nc.gpsimd.dma_start(out=bias_sb, in_=biasb[:])

            def mm_tile(h_sb, ps, c, start, stop):
                lhsT = h_sb[:, c * 128:(c + 1) * 128]
                nc.tensor.matmul(
                    ps[:, 0:512], lhsT, wt_sb[c][:, 0:512],
                    start=start, stop=stop,
                )
                nc.tensor.matmul(
                    ps[:, 512:E], lhsT, wt_sb[c][:, 512:E],
                    start=start, stop=stop,
                )

            # All per-tile results accumulate in one resident SBUF buffer and
            # leave in a single big DMA after the last tile: per-tile 132 B/
            # partition DMAs have poor efficiency, and an output DMA queued on
            # a busy engine serializes the next tile's post chain behind its
            # DVE-completion wait.
            comb_all = cpool.tile([128, TT * 33], F32, name="comb_all")

            def post_tile(t, ps, z_engine):
                # packed result slice: z16 | idx16 | se
                comb = comb_all[:, t * 33:(t + 1) * 33]
                se = comb[:, 32:33]

                # e = exp(logits), se = rowsum(e)  (ScalarE, single pass)
                ez = mpool.tile([128, E], F32, tag="ez")
                nc.scalar.activation(out=ez, in_=ps, func=EXP, accum_out=se)

                # z = bias * se + e — scaled-bias on ScalarE (AP scale), add on
                # gpsimd in steady state / DVE for the last tile (tail latency)
                br = mpool.tile([128, E], F32, tag="br")
                nc.scalar.activation(
                    out=br, in_=bias_sb, func=mybir.ActivationFunctionType.Copy,
                    scale=se,
                )
                z = mpool.tile([128, E], F32, tag="z")
                z_engine.tensor_add(z, ez, br)

                # top-16 (values + indices), descending
                i16 = comb[:, 16:32].bitcast(U32)
                z2 = mpool.tile([128, E], F32, tag="z2")
                nc.vector.max(comb[:, 0:8], z)
                nc.vector.max_index(i16[:, 0:8], comb[:, 0:8], z)
                nc.vector.match_replace(z2, comb[:, 0:8], z, imm_value=-1.0)
                nc.vector.max(comb[:, 8:16], z2)
                nc.vector.max_index(i16[:, 8:16], comb[:, 8:16], z2)

            # ---- prologue: greedy (tile, chunk) order over PRO_T tiles ------
            ps_pro = [
                ppool.tile([128, E], F32, tag="ps", name=f"ps_pro{i}")
                for i in range(PRO_T)
            ]
            seen = {t: 0 for t in range(PRO_T)}
            order = _prologue_order()
            for t, c in order:
                seen[t] += 1
                mm_tile(h_tiles[t], ps_pro[t], c,
                        start=(seen[t] == 1), stop=(seen[t] == KC))
            for t in range(PRO_T):
                post_tile(t, ps_pro[t], nc.gpsimd)

            # ---- steady state: tile-major -----------------------------------
            for t in range(PRO_T, TT):
                h_sb = hpool.tile([128, KC * 128], mm_dtype, tag="h")
                nc.sync.dma_start(out=h_sb, in_=ht[t])
                ps = ppool.tile([128, E], F32, tag="ps")
                for c in range(KC):
                    mm_tile(h_sb, ps, c, start=(c == 0), stop=(c == KC - 1))
                z_eng = nc.vector if t == TT - 1 else nc.gpsimd
                post_tile(t, ps, z_eng)
                if t == TT - 5:
                    # bulk of the output leaves mid-stream; only the last few
                    # tiles' slices ride the (latency-sensitive) final DMA
                    nc.gpsimd.dma_start(
                        out=o_pack[:, 0:(TT - 4) * 33],
                        in_=comb_all[:, 0:(TT - 4) * 33],
                    )
            # final slice rides the scalar queue: idle by now, and its HWDGE
            # completion (and drain) beats gpsimd's SWDGE
            nc.scalar.dma_start(
                out=o_pack[:, (TT - 4) * 33:], in_=comb_all[:, (TT - 4) * 33:]
            )
    nc.finalize()
    return nc


def _prep_inputs(h, W_, b, np_dtype):
    # [k_in_chunk(p), chunk(c), expert(e)]: wtprep[p, c, e] = W[e, c*128 + p]
    wtprep = np.ascontiguousarray(
        W_.T.reshape(KC, 128, E).transpose(1, 0, 2).astype(np_dtype)
    )
    biasb = np.ascontiguousarray(np.broadcast_to(b, (128, E)).astype(np.float32))
    in_maps = []
    for core in range(N_CORES):
        hc = h[core * TOK:(core + 1) * TOK]
        # [tile, token_in_tile(j), chunk(c), k_in_chunk(p)] -> [tile, p, c, j]
        h4 = hc.reshape(TT, 128, KC, 128)
        htp = np.ascontiguousarray(h4.transpose(0, 3, 2, 1).astype(np_dtype))
        in_maps.append({"ht": htp, "wt": wtprep, "biasb": biasb})
    return in_maps


RISK_TAU = 1e-3  # relative z-gap below which matmul noise could flip ordering
N_RISK_GAPS = 12  # only gaps among ranks 0..12 affect the top-12 output


def _epilogue(results, b, h_flat, W):
    idx_list, w_list, risk_list = [], [], []
    for r in results:
        # o_pack is [128 token-in-tile, TT*33]: record for token t*128+p is
        # at [p, t*33:(t+1)*33]
        pack = np.ascontiguousarray(
            r["o_pack"].reshape(128, TT, 33).transpose(1, 0, 2).reshape(-1, 33)
        )
        z16 = pack[:, 0:16]
        idx16 = pack[:, 16:32].view(np.uint32)
        se = pack[:, 32:33]
        e16 = (z16 - se * b[idx16]).astype(np.float32)
        e12 = e16[:, :TOPK]
        denom = e12.sum(axis=-1, keepdims=True, dtype=np.float32) + np.float32(1e-20) * se
        w_list.append((np.float32(SCALE) * e12 / denom).astype(np.float32))
        idx_list.append(idx16[:, :TOPK].astype(np.int32))
        # flag tokens whose output-relevant top-13 gaps are inside the noise band
        gaps = (z16[:, :N_RISK_GAPS] - z16[:, 1:N_RISK_GAPS + 1]) / np.abs(z16[:, :1])
        risk_list.append(gaps.min(axis=-1) < RISK_TAU)
    topk_idx = np.concatenate(idx_list, axis=0)
    topk_w = np.concatenate(w_list, axis=0)

    # fp32-exact host recompute for at-risk tokens (mimics the reference op
    # sequence exactly in float32)
    risk = np.concatenate(risk_list, axis=0)
    ridx = np.nonzero(risk)[0]
    if ridx.size:
        lg = h_flat[ridx] @ W.T.astype(np.float32)
        mx = lg.max(axis=-1, keepdims=True)
        ex = np.exp(lg - mx)
        s = ex / ex.sum(axis=-1, keepdims=True, dtype=np.float32)
        c = s + b
        ii = np.argsort(-c, axis=-1, kind="stable")[:, :TOPK]
        ww = np.take_along_axis(s, ii, axis=-1)
        ww = ww / (ww.sum(axis=-1, keepdims=True, dtype=np.float32) + np.float32(1e-20))
        topk_idx[ridx] = ii.astype(np.int32)
        topk_w[ridx] = (np.float32(SCALE) * ww).astype(np.float32)

    topk_idx = topk_idx.reshape(B, S, TOPK)
    topk_w = topk_w.reshape(B, S, TOPK).astype(np.float32)
    return topk_idx, topk_w


_NC_CACHE = {}


def run(hidden_states, W, e_score_correction_bias, trace=False, mm_dtype=F16):
    key = (str(mm_dtype),)
    if key not in _NC_CACHE:
        _NC_CACHE[key] = build_nc(mm_dtype)
    nc = _NC_CACHE[key]
    np_dtype = np.float16 if mm_dtype == F16 else np.float32
    h = np.ascontiguousarray(np.asarray(hidden_states, dtype=np.float32)).reshape(-1, H)
    W_ = np.ascontiguousarray(np.asarray(W, dtype=np.float32))
    b = np.ascontiguousarray(np.asarray(e_score_correction_bias, dtype=np.float32))
    in_maps = _prep_inputs(h, W_, b, np_dtype)
    res = run_bass_kernel_spmd(nc, in_maps, core_ids=list(range(N_CORES)), trace=trace)
    out = _epilogue(res.results, b, h, W_)
    return out, res


def kernel(hidden_states, W, e_score_correction_bias):
    out, _ = run(hidden_states, W, e_score_correction_bias, trace=False)
    return out


# revision 33
# speedup vs baseline: 1.0036x; 1.0036x over previous
"""LongcatFlashTopkRouter on 8 Trainium2 NeuronCores.

Math (per token t):
    logits = h_t @ W.T                      # [768]
    s      = softmax(logits)
    c      = s + bias                       # bias-corrected selection scores
    idx    = top12(c)                       # descending, ties -> lower index
    w      = 2.5 * s[idx] / sum(s[idx])

Device-side reformulation (per token, no softmax materialization needed):
    e   = exp(logits)           (no max-subtraction: |logits| < ~9 is safe in fp32)
    se  = sum(e)
    z   = e + se * bias         # z has the SAME ordering as c = e/se + bias
    top-16 of z -> (z16, idx16) via DVE max/max_index/match_replace
Host epilogue (cheap, vectorized numpy):
    e16 = z16 - se * bias[idx16]
    w   = 2.5 * e16[:, :12] / sum(e16[:, :12])   # the 1/se factor cancels
Tokens whose top-13 adjacent z-gaps are inside the matmul noise band are
recomputed exactly on the host (same op sequence as the reference).

Matmul runs in fp16 (1 cycle/row on the PE like fp32r, but FWL weight loads
are enabled — fp32-HIGH mode disables them — and DMA bytes halve, which
removes the early-kernel HBM oversubscription from the 6 MB weight preload).
fp16 logit noise (~2.7e-4 std) sits 4x under the 1e-3 risk threshold.

Sharding: tokens (batch*seq = 32768) split evenly across 8 cores (4096 each);
W and bias replicated. Hidden states are pre-transposed on the host into
[tile, k-partition, k-chunk, token] layout so each 128-token tile's 16
contraction chunks are contiguous SBUF-ready blocks.
"""

import numpy as np

import concourse.bass as bass
import concourse.mybir as mybir
from concourse import bacc
from concourse.tile import TileContext
from concourse.bass_utils import run_bass_kernel_spmd

N_CORES = 8
B, S, H, E = 4, 8192, 2048, 768
TOK = B * S // N_CORES      # 4096 tokens per core
TT = 32                     # token tiles of 128 per core
KC = H // 128               # 16 contraction chunks
TOPK = 12
TOP16 = 16
SCALE = 2.5

F32 = mybir.dt.float32
F16 = mybir.dt.float16
F32R = mybir.dt.float32r
U32 = mybir.dt.uint32
EXP = mybir.ActivationFunctionType.Exp
MULT = mybir.AluOpType.mult
ADD = mybir.AluOpType.add

PRO_T = 4  # prologue tiles processed chunk-major while wt streams in


def _prologue_order():
    """Static greedy order of (tile, chunk) matmul pairs for the prologue.

    Uses estimated DMA arrival times (us, relative to first transfer):
    h0/h1 land as two half-tile pieces, h2/h3 whole; wt chunks stream on
    their own queue. Greedy: run whichever pair is ready, else wait for
    the earliest one.
    """
    pair = 0.34
    # Queues are FIFO at ~95 GB/s each: sync = [wt0, h0(c0-3), h0(c4-15)],
    # scalar = [h1, h2, h3], gpsimd = [wt1..wt15] at ~0.95us/chunk.
    # Arrival estimates relative to DMA flow start (~7.4us absolute).
    h_avail = {
        (0, 0): 3.3, (0, 1): 4.3,   # (tile0, chunks 0-7 / 8-15)
        (1, 0): 4.6, (1, 1): 4.6,
        (2, 0): 6.5, (2, 1): 6.5,
        (3, 0): 6.2, (3, 1): 6.2,
    }
    # wt0 leads on the (fast-waking) sync queue; wt1+ ride gpsimd whose first
    # packet flows ~4us in
    wt_avail = [2.3] + [4.0 + 0.72 * c for c in range(1, KC)]
    remaining = [(t, c) for t in range(PRO_T) for c in range(KC)]
    ready_at = {
        (t, c): max(h_avail[(t, 0 if c < 8 else 1)], wt_avail[c])
        for t, c in remaining
    }
    order = []
    pe_t = 0.0
    while remaining:
        ready = [p for p in remaining if ready_at[p] <= pe_t]
        pick = min(ready, key=lambda p: (ready_at[p], p[1], p[0])) if ready else \
            min(remaining, key=lambda p: (ready_at[p], p[1], p[0]))
        remaining.remove(pick)
        order.append(pick)
        pe_t = max(pe_t, ready_at[pick]) + pair
    return order


def build_nc(mm_dtype=F16):
    nc = bacc.Bacc()
    ht = nc.dram_tensor("ht", [TT, 128, KC, 128], mm_dtype, kind="ExternalInput")
    wt = nc.dram_tensor("wt", [128, KC, E], mm_dtype, kind="ExternalInput")
    biasb = nc.dram_tensor("biasb", [128, E], F32, kind="ExternalInput")
    # packed output, one 33-f32 record per (token-in-tile, tile):
    # [z16 f32 | idx16 u32-bits | sumexp f32]; written in ONE end-of-kernel
    # DMA (per-tile 132 B/partition DMAs have terrible completion latency)
    o_pack = nc.dram_tensor("o_pack", [128, TT * 33], F32, kind="ExternalOutput")

    with TileContext(nc) as tc:
        with (
            tc.tile_pool(name="const", bufs=1) as cpool,
            tc.tile_pool(name="hin", bufs=8) as hpool,
            tc.tile_pool(name="mid", bufs=4) as mpool,
            tc.tile_pool(name="ps", bufs=PRO_T, space="PSUM") as ppool,
        ):
            # ---- input DMAs -------------------------------------------------
            # Prologue h tiles split across the sync and scalar queues so they
            # land in parallel (steady-state h rides sync alone: ~82 GB/s
            # demand vs ~260 GB/s the queue sustains). h0 lands as two
            # half-tile pieces so the first matmuls can start early.
            h_tiles = {}
            for t in range(PRO_T):
                h_tiles[t] = hpool.tile(
                    [128, KC * 128], mm_dtype, tag="h", name=f"h_p{t}"
                )
            # wt chunk 0 leads on the sync queue: each DMA queue pays a ~2-3us
            # cold-start before its first packet flows, and gpsimd's is the
            # slowest — wt0 there would gate the first matmul. Remaining wt
            # chunks + bias stream on gpsimd, one tile per chunk so a matmul
            # only waits on its own chunk.
            wt_sb = []
            for c in range(KC):
                wt_sb.append(
                    cpool.tile([128, E], mm_dtype, tag=f"wt{c}", name=f"wt_c{c}")
                )
            nc.sync.dma_start(out=wt_sb[0], in_=wt[:, 0])
            half = KC * 128 // 2
            nc.sync.dma_start(out=h_tiles[0][:, 0:half], in_=ht[0, :, 0:KC // 2])
            nc.sync.dma_start(out=h_tiles[0][:, half:], in_=ht[0, :, KC // 2:])
            nc.scalar.dma_start(out=h_tiles[1], in_=ht[1])
            nc.sync.dma_start(out=h_tiles[3], in_=ht[3])
            nc.scalar.dma_start(out=h_tiles[2], in_=ht[2])
            for c in range(1, KC):
                nc.gpsimd.dma_start(out=wt_sb[c], in_=wt[:, c])
            bias_sb = cpool.tile([128, E], F32)
            nc.gpsimd.dma_start(out=bias_sb, in_=biasb[:])

            def mm_tile(h_sb, ps, c, start, stop):
                lhsT = h_sb[:, c * 128:(c + 1) * 128]
                nc.tensor.matmul(
                    ps[:, 0:512], lhsT, wt_sb[c][:, 0:512],
                    start=start, stop=stop,
                )
                nc.tensor.matmul(
                    ps[:, 512:E], lhsT, wt_sb[c][:, 512:E],
                    start=start, stop=stop,
                )

            # All per-tile results accumulate in one resident SBUF buffer and
            # leave in a single big DMA after the last tile: per-tile 132 B/
            # partition DMAs have poor efficiency, and an output DMA queued on
            # a busy engine serializes the next tile's post chain behind its
            # DVE-completion wait.
            comb_all = cpool.tile([128, TT * 33], F32, name="comb_all")

            def post_tile(t, ps, z_engine):
                # packed result slice: z16 as fp16 (8 f32 cols) | pad | idx16 | se
                comb = comb_all[:, t * 33:(t + 1) * 33]
                se = comb[:, 32:33]

                # e = exp(logits), se = rowsum(e)  (ScalarE, single pass)
                ez = mpool.tile([128, E], F32, tag="ez")
                nc.scalar.activation(out=ez, in_=ps, func=EXP, accum_out=se)

                # z = bias * se + e — scaled-bias on ScalarE (AP scale), add on
                # gpsimd in steady state / DVE for the last tile (tail latency).
                # z is produced in fp16: rounding is monotone (never reorders,
                # only ties, which the host's gap flag catches) and 16-bit
                # doubles DVE top-k throughput. Weight VALUES don't come from
                # z anymore — the host recomputes the 12 selected logits
                # exactly, so only the ordering and the risk gaps need z.
                br = mpool.tile([128, E], F32, tag="br")
                nc.scalar.activation(
                    out=br, in_=bias_sb, func=mybir.ActivationFunctionType.Copy,
                    scale=se,
                )
                zh = mpool.tile([128, E], F16, tag="z")
                z_engine.tensor_add(zh, ez, br)

                # top-16 (fp16 values + u32 indices), descending
                z16a = comb[:, 0:4].bitcast(F16)
                z16b = comb[:, 4:8].bitcast(F16)
                i16 = comb[:, 16:32].bitcast(U32)
                z2 = mpool.tile([128, E], F16, tag="z2")
                nc.vector.max(z16a, zh)
                nc.vector.max_index(i16[:, 0:8], z16a, zh)
                nc.vector.match_replace(z2, z16a, zh, imm_value=-1.0)
                nc.vector.max(z16b, z2)
                nc.vector.max_index(i16[:, 8:16], z16b, z2)

            # ---- prologue: greedy (tile, chunk) order over PRO_T tiles ------
            ps_pro = [
                ppool.tile([128, E], F32, tag="ps", name=f"ps_pro{i}")
                for i in range(PRO_T)
            ]
            seen = {t: 0 for t in range(PRO_T)}
            order = _prologue_order()
            for t, c in order:
                seen[t] += 1
                mm_tile(h_tiles[t], ps_pro[t], c,
                        start=(seen[t] == 1), stop=(seen[t] == KC))
            for t in range(PRO_T):
                post_tile(t, ps_pro[t], nc.gpsimd)

            # ---- steady state: tile-major -----------------------------------
            for t in range(PRO_T, TT):
                h_sb = hpool.tile([128, KC * 128], mm_dtype, tag="h")
                nc.sync.dma_start(out=h_sb, in_=ht[t])
                ps = ppool.tile([128, E], F32, tag="ps")
                for c in range(KC):
                    mm_tile(h_sb, ps, c, start=(c == 0), stop=(c == KC - 1))
                z_eng = nc.vector if t == TT - 1 else nc.gpsimd
                post_tile(t, ps, z_eng)
                if t == TT - 5:
                    # bulk of the output leaves mid-stream; only the last few
                    # tiles' slices ride the (latency-sensitive) final DMA
                    nc.gpsimd.dma_start(
                        out=o_pack[:, 0:(TT - 4) * 33],
                        in_=comb_all[:, 0:(TT - 4) * 33],
                    )
            # final slice rides the scalar queue: idle by now, and its HWDGE
            # completion (and drain) beats gpsimd's SWDGE
            nc.scalar.dma_start(
                out=o_pack[:, (TT - 4) * 33:], in_=comb_all[:, (TT - 4) * 33:]
            )
    nc.finalize()
    return nc


def _prep_inputs(h, W_, b, np_dtype):
    # [k_in_chunk(p), chunk(c), expert(e)]: wtprep[p, c, e] = W[e, c*128 + p]
    wtprep = np.ascontiguousarray(
        W_.T.reshape(KC, 128, E).transpose(1, 0, 2).astype(np_dtype)
    )
    biasb = np.ascontiguousarray(np.broadcast_to(b, (128, E)).astype(np.float32))
    in_maps = []
    for core in range(N_CORES):
        hc = h[core * TOK:(core + 1) * TOK]
        # [tile, token_in_tile(j), chunk(c), k_in_chunk(p)] -> [tile, p, c, j]
        h4 = hc.reshape(TT, 128, KC, 128)
        htp = np.ascontiguousarray(h4.transpose(0, 3, 2, 1).astype(np_dtype))
        in_maps.append({"ht": htp, "wt": wtprep, "biasb": biasb})
    return in_maps


# relative z-gap below which matmul noise (p99.9 ~2.3e-4) plus fp16-z
# quantization (+-1 ulp = 4.9e-4 on the gap measurement) could flip ordering;
# only gaps among ranks 0..12 affect the top-12 output
RISK_TAU = 1.2e-3
N_RISK_GAPS = 12


def _epilogue(results, b, h_flat, W):
    idx_list, z_list = [], []
    for r in results:
        # o_pack is [128 token-in-tile, TT*33]: record for token t*128+p is
        # at [p, t*33:(t+1)*33]
        pack = np.ascontiguousarray(
            r["o_pack"].reshape(128, TT, 33).transpose(1, 0, 2).reshape(-1, 33)
        )
        z16 = np.ascontiguousarray(pack[:, 0:8]).view(np.float16).astype(np.float32)
        idx16 = pack[:, 16:32].view(np.uint32)
        idx_list.append(idx16[:, :TOPK].astype(np.int32))
        z_list.append(z16)
    topk_idx = np.concatenate(idx_list, axis=0)
    z16 = np.concatenate(z_list, axis=0)

    # flag tokens whose output-relevant top-13 gaps are inside the noise band
    gaps = (z16[:, :N_RISK_GAPS] - z16[:, 1:N_RISK_GAPS + 1]) / np.abs(z16[:, :1])
    risk = gaps.min(axis=-1) < RISK_TAU

    n = topk_idx.shape[0]
    topk_w = np.empty((n, TOPK), np.float32)

    # unflagged tokens: indices are trusted; recompute the 12 selected logits
    # exactly (the softmax denominator cancels in the normalized weights, so
    # 12 dot products per token give the reference weights to fp32 accuracy)
    uidx = np.nonzero(~risk)[0]
    if uidx.size:
        hu = h_flat[uidx]
        iu = topk_idx[uidx]
        lg = np.empty((uidx.size, TOPK), np.float32)
        for j in range(TOPK):
            lg[:, j] = np.einsum("mk,mk->m", hu, W[iu[:, j]])
        ex = np.exp(lg - lg.max(axis=-1, keepdims=True))
        topk_w[uidx] = (
            np.float32(SCALE) * ex / ex.sum(axis=-1, keepdims=True, dtype=np.float32)
        ).astype(np.float32)

    # fp32-exact host recompute for at-risk tokens (mimics the reference op
    # sequence exactly in float32)
    ridx = np.nonzero(risk)[0]
    if ridx.size:
        lg = h_flat[ridx] @ W.T.astype(np.float32)
        mx = lg.max(axis=-1, keepdims=True)
        ex = np.exp(lg - mx)
        s = ex / ex.sum(axis=-1, keepdims=True, dtype=np.float32)
        c = s + b
        ii = np.argsort(-c, axis=-1, kind="stable")[:, :TOPK]
        ww = np.take_along_axis(s, ii, axis=-1)
        ww = ww / (ww.sum(axis=-1, keepdims=True, dtype=np.float32) + np.float32(1e-20))
        topk_idx[ridx] = ii.astype(np.int32)
        topk_w[ridx] = (np.float32(SCALE) * ww).astype(np.float32)

    topk_idx = topk_idx.reshape(B, S, TOPK)
    topk_w = topk_w.reshape(B, S, TOPK).astype(np.float32)
    return topk_idx, topk_w


_NC_CACHE = {}


def run(hidden_states, W, e_score_correction_bias, trace=False, mm_dtype=F16):
    key = (str(mm_dtype),)
    if key not in _NC_CACHE:
        _NC_CACHE[key] = build_nc(mm_dtype)
    nc = _NC_CACHE[key]
    np_dtype = np.float16 if mm_dtype == F16 else np.float32
    h = np.ascontiguousarray(np.asarray(hidden_states, dtype=np.float32)).reshape(-1, H)
    W_ = np.ascontiguousarray(np.asarray(W, dtype=np.float32))
    b = np.ascontiguousarray(np.asarray(e_score_correction_bias, dtype=np.float32))
    in_maps = _prep_inputs(h, W_, b, np_dtype)
    res = run_bass_kernel_spmd(nc, in_maps, core_ids=list(range(N_CORES)), trace=trace)
    out = _epilogue(res.results, b, h, W_)
    return out, res


def kernel(hidden_states, W, e_score_correction_bias):
    out, _ = run(hidden_states, W, e_score_correction_bias, trace=False)
    return out
